# revision 35
# baseline (speedup 1.0000x reference)
"""Trainium2 Bass kernel for nn_RandNLAGQALayer (sparse attention, GQA, kron-sketch).

Sharding: 8-way tensor-parallel over heads. Each core computes 2 q-heads and its
(duplicated across a pair) kv head, full sequence. Importance scorer + top-k
threshold + sketch are computed per-core (replicated, tiny). Output projection
partial sums are reduced on host.
"""

import math
import numpy as np

import concourse.bass as bass
import concourse.mybir as mybir
import concourse.tile as tile
from concourse.tile import TileContext, ScopedClock
from concourse.masks import make_identity
from concourse.bass_utils import run_bass_kernel_spmd

F32 = mybir.dt.float32
BF16 = mybir.dt.bfloat16
AX = mybir.AxisListType
OP = mybir.AluOpType
AF = mybir.ActivationFunctionType

B, S, HID = 1, 4096, 2048
NH, NKV, HD = 16, 4, 128
SKETCH, TOPK = 640, 2048
KA_R, KA_C = 20, 128
KB_R, KB_C = 32, 256
EPS = 1e-6
NCORES = 8
P = 128
NST = S // P            # 32 s-tiles
NHT = HID // P          # 16 h-tiles
NSB = S // 512          # 8 sq blocks
TSK = 80                # sketch keys that can ever be unmasked (t*51.2<=4095)
ISQ = 1.0 / math.sqrt(HD)

def _split_sync_waits(nc, maxw=1):
    """This container's walrus rejects instructions carrying more than ~1 sync
    wait. Move extra waits onto same-engine nops inserted just before."""
    for f in nc.m.functions:
        for bb in f.blocks:
            newlist = []
            for ins in bb.instructions:
                si = ins.sync_info
                waits = list(si.on_wait) if si and si.on_wait else []
                if len(waits) > maxw:
                    si.on_wait = waits[:maxw]
                    for w in waits[maxw:]:
                        nop = mybir.InstNoOp(name=f"I-{nc.next_id()}", ins=[], outs=[])
                        nop.engine = ins.engine
                        nop.sync_info = mybir.SyncInfo(on_wait=[w], on_update=[])
                        nc.register_instruction(nop, overwrite=True)
                        newlist.append(nop)
                newlist.append(ins)
            bb.instructions[:] = newlist


def build_program(debug=False):
    nc = bass.Bass("TRN2", target_bir_lowering=False, debug=False, num_devices=NCORES)

    def din(name, shape):
        return nc.dram_tensor(name, shape, F32, kind="ExternalInput")

    x = din("x", [S, HID])
    wqkv = din("wqkv", [HID, 512])
    wo = din("wo", [256, HID])
    w1 = din("w1", [HID, 64])
    w2 = din("w2", [64, 1])
    b1c = din("b1c", [64, 1])
    b2c = din("b2c", [1, 1])
    qknormw = din("qknormw", [1, 384])
    skscale = din("skscale", [1, 1])
    cosj = din("cosj", [P, NST * 64])
    sinj = din("sinj", [P, NST * 64])
    bj = din("bj", [P, NST * TSK])
    mask_sk = nc.dram_tensor("mask_sk", [P, S], BF16, kind="ExternalInput")
    tri = nc.dram_tensor("tri", [P, 4 * 512], BF16, kind="ExternalInput")
    iota1 = din("iota1", [P, 1])
    onescol = din("onescol", [P, 1])
    ones1r = din("ones1r", [1, P])
    ones128 = nc.dram_tensor("ones128", [P, P], BF16, kind="ExternalInput")

    ypart = nc.dram_tensor("ypart", [HID, S], F32, kind="ExternalOutput")
    ldram = nc.dram_tensor("ldram", [1, S], F32)

    dbg = {}
    if debug:
        for nm, shp in [
            ("d_logits", [P, NST]), ("d_tau", [P, 1]), ("d_restw", [P, NST]),
            ("d_selb", [P, NST]),
        ]:
            dbg[nm] = nc.dram_tensor(nm, shp, F32, kind="ExternalOutput")
        for nm, shp in [
            ("d_ksk", [P, TSK]), ("d_vsk", [TSK, P]),
            ("d_qt", [P, S]), ("d_kt", [P, S]), ("d_v", [P, S]),
            ("d_o", [P, 2 * S]),
        ]:
            dbg[nm] = nc.dram_tensor(nm, shp, BF16, kind="ExternalOutput")

    from contextlib import ExitStack
    with TileContext(nc) as tc, ExitStack() as es:
        cpool = es.enter_context(tc.tile_pool(name="consts", bufs=1))
        big = es.enter_context(tc.tile_pool(name="big", bufs=1))

        ident = cpool.tile([P, P], BF16)
        make_identity(nc, ident[:])
        iota1_sb = cpool.tile([P, 1], F32)
        nc.sync.dma_start(out=iota1_sb[:], in_=iota1[:])
        onescol_sb = cpool.tile([P, 1], F32)
        nc.sync.dma_start(out=onescol_sb[:], in_=onescol[:])
        ones1r_sb = cpool.tile([1, P], F32)
        nc.sync.dma_start(out=ones1r_sb[:], in_=ones1r[:])
        ones128_sb = cpool.tile([P, P], BF16)
        nc.sync.dma_start(out=ones128_sb[:], in_=ones128[:])
        qknw_sb = cpool.tile([P, 384], F32)
        nc.sync.dma_start(out=qknw_sb[:], in_=qknormw[:].to_broadcast((P, 384)))
        b2c_sb = cpool.tile([P, 1], F32)
        nc.sync.dma_start(out=b2c_sb[:], in_=b2c[:].to_broadcast((P, 1)))
        sksc_sb = cpool.tile([P, 1], F32)
        nc.sync.dma_start(out=sksc_sb[:], in_=skscale[:].to_broadcast((P, 1)))
        b1c_sb = cpool.tile([64, 1], F32)
        nc.sync.dma_start(out=b1c_sb[:], in_=b1c[:])

        # persistent big tensors
        qt = big.tile([P, 2 * S], BF16)          # q^T roped, head h at [S*h : ...]
        kt = big.tile([P, S], BF16)              # k^T roped
        vsb = big.tile([P, S], BF16)             # v in [s,d], s-tile j at cols [128j:...]
        lg_stuff = es.enter_context(tc.tile_pool(name="lgp", bufs=1))
        logits_pm = lg_stuff.tile([P, NST], F32)
        selbias_pm = lg_stuff.tile([P, NST], F32)
        restw_pm = lg_stuff.tile([P, NST], F32)
        sk_stuff = es.enter_context(tc.tile_pool(name="skp", bufs=1))
        kskT_sb = sk_stuff.tile([P, TSK], BF16)
        vsk_sb = sk_stuff.tile([TSK, P], BF16)
        xtpool_cm = tc.tile_pool(name="xtp", bufs=1)
        xtpool = xtpool_cm.__enter__()
        xt = xtpool.tile([P, NHT * S], BF16)     # x^T, h-tile ht at cols [S*ht : S*(ht+1)]

        # ---- stage 0: cast x to bf16 hi+lo (via DRAM), then transposed loads ----
        xbf = nc.dram_tensor("xbf", [S, HID], BF16)
        xlo_d = nc.dram_tensor("xlo", [S, HID], BF16)
        with tc.tile_pool(name="xload", bufs=2) as xp:
            for j in range(NST):
                xf = xp.tile([P, HID], F32, tag="xf")
                nc.sync.dma_start(out=xf[:], in_=x[P * j : P * (j + 1), :])
                xb = xp.tile([P, HID], BF16, tag="xb")
                nc.any.tensor_copy(xb[:], xf[:])
                nc.sync.dma_start(out=xbf[P * j : P * (j + 1), :], in_=xb[:])
                xl = xp.tile([P, HID], BF16, tag="xl")
                nc.vector.tensor_tensor(xl[:], xf[:], xb[:], op=OP.subtract)
                nc.sync.dma_start(out=xlo_d[P * j : P * (j + 1), :], in_=xl[:])
            for ht in range(NHT):
                nc.sync.dma_start(out=xt[:, S * ht : S * (ht + 1)],
                                  in_=xbf[:, P * ht : P * (ht + 1)], transpose=True)

        # ---- stage 1: logits + threshold + rest_w ----
        with tc.tile_pool(name="lgA", bufs=1) as lp, \
             tc.tile_pool(name="lgps", bufs=1, space="PSUM") as lps:
            w1f = lp.tile([P, NHT * 64], F32)
            nc.sync.dma_start(out=w1f[:].rearrange("p (t f) -> p t f", f=64),
                              in_=w1[:].rearrange("(t p) f -> p t f", p=P))
            w1b = lp.tile([P, NHT * 64], BF16)
            nc.any.tensor_copy(w1b[:], w1f[:])
            w1lo = lp.tile([P, NHT * 64], BF16)
            nc.vector.tensor_tensor(w1lo[:], w1f[:], w1b[:], op=OP.subtract)
            w2f = lp.tile([64, 1], F32)
            nc.sync.dma_start(out=w2f[:], in_=w2[:])

            # u = x@W1 in split bf16 precision (hi*hi + hi*lo + lo*hi)
            tt_sb = lp.tile([64, S], F32)
            with tc.tile_pool(name="xlo_t", bufs=2) as xlp:
                for grp in range(2):
                    tps = []
                    for i in range(4):
                        lg_u_ps = lps.tile([64, 512], F32, tag=f"lgps{i}")
                        tps.append(lg_u_ps)
                    for ht in range(NHT):
                        xlt = xlp.tile([P, S], BF16)
                        nc.sync.dma_start(out=xlt[:],
                                          in_=xlo_d[:, P * ht : P * (ht + 1)],
                                          transpose=True)
                        for i in range(4):
                            sb = 4 * grp + i
                            sl = slice(S * ht + 512 * sb, S * ht + 512 * (sb + 1))
                            nc.tensor.matmul(tps[i][:],
                                             lhsT=w1b[:, 64 * ht : 64 * (ht + 1)],
                                             rhs=xt[:, sl], start=(ht == 0), stop=False)
                            nc.tensor.matmul(tps[i][:],
                                             lhsT=w1lo[:, 64 * ht : 64 * (ht + 1)],
                                             rhs=xt[:, sl], start=False, stop=False)
                            nc.tensor.matmul(tps[i][:],
                                             lhsT=w1b[:, 64 * ht : 64 * (ht + 1)],
                                             rhs=xlt[:, 512 * sb : 512 * (sb + 1)],
                                             start=False, stop=(ht == NHT - 1))
                    for i in range(4):
                        sb = 4 * grp + i
                        nc.scalar.activation(tt_sb[:, 512 * sb : 512 * (sb + 1)],
                                             tps[i][:], AF.Tanh, bias=b1c_sb[:])

            lg_ps = lps.tile([P, NST], F32)
            for j in range(NST):
                nc.tensor.matmul(lg_ps[:, j : j + 1], lhsT=tt_sb[:, P * j : P * (j + 1)],
                                 rhs=w2f[:], start=True, stop=True)
            nc.vector.tensor_scalar(logits_pm[:], lg_ps[:], b2c_sb[:, 0:1], None,
                                    op0=OP.add)

        with tc.tile_pool(name="lgB", bufs=1) as lp, \
             tc.tile_pool(name="lgBps", bufs=2, space="PSUM") as lps:
            # threshold search
            nc.sync.dma_start(out=ldram[0:1, :].rearrange("o (j p) -> o p j", p=P),
                              in_=logits_pm[:])
            lrep = lp.tile([P, S], F32)
            nc.sync.dma_start(out=lrep[:], in_=ldram[:].to_broadcast((P, S)))
            lo = lp.tile([P, 1], F32)
            wd = lp.tile([P, 1], F32)
            nc.vector.memset(lo[:], -6.0)
            nc.vector.memset(wd[:], 8.0 / 128.0)
            cmp = lp.tile([P, S], BF16)
            thr = lp.tile([P, 1], F32)
            cnt = lp.tile([P, 1], F32)
            ge = lp.tile([P, 1], F32)
            nselb = lp.tile([P, 1], F32)
            nsel_sb = lp.tile([1, 1], F32)
            for lvl in range(4):
                nc.vector.tensor_tensor(thr[:], iota1_sb[:], wd[:], op=OP.mult)
                nc.vector.tensor_tensor(thr[:], thr[:], lo[:], op=OP.add)
                nc.vector.tensor_scalar(cmp[:], lrep[:], thr[:, 0:1], None, op0=OP.is_gt)
                nc.vector.tensor_reduce(cnt[:], cmp[:], axis=AX.X, op=OP.add)
                nc.vector.tensor_scalar(ge[:], cnt[:], 2047.5, None, op0=OP.is_ge)
                ns_ps = lps.tile([1, 1], F32)
                nc.tensor.matmul(ns_ps[:], lhsT=onescol_sb[:], rhs=ge[:],
                                 start=True, stop=True)
                nc.any.tensor_copy(nsel_sb[:], ns_ps[:])
                nsb_ps = lps.tile([P, 1], F32)
                nc.tensor.matmul(nsb_ps[:], lhsT=ones1r_sb[:], rhs=nsel_sb[:],
                                 start=True, stop=True)
                nc.any.tensor_copy(nselb[:], nsb_ps[:])
                nc.vector.tensor_tensor(nselb[:], nselb[:], wd[:], op=OP.mult)
                nc.vector.tensor_tensor(lo[:], lo[:], nselb[:], op=OP.add)
                if lvl < 3:
                    nc.vector.tensor_scalar(wd[:], wd[:], 1.0 / 128.0, None, op0=OP.mult)

            nc.vector.tensor_scalar(selbias_pm[:], logits_pm[:], lo[:, 0:1], -1e4,
                                    op0=OP.is_le, op1=OP.mult)
            iw_pm = lp.tile([P, NST], F32)
            nc.scalar.activation(iw_pm[:], logits_pm[:], AF.Sigmoid)
            nc.vector.tensor_scalar(restw_pm[:], logits_pm[:], lo[:, 0:1], None,
                                    op0=OP.is_le)
            nc.vector.tensor_tensor(restw_pm[:], restw_pm[:], iw_pm[:], op=OP.mult)
            if debug:
                nc.sync.dma_start(out=dbg["d_logits"][:], in_=logits_pm[:])
                dtau = lp.tile([P, 1], F32)
                nc.any.tensor_copy(dtau[:], lo[:])
                nc.sync.dma_start(out=dbg["d_tau"][:], in_=dtau[:])
                nc.sync.dma_start(out=dbg["d_restw"][:], in_=restw_pm[:])
                nc.sync.dma_start(out=dbg["d_selb"][:], in_=selbias_pm[:])

        # ---- stage 2: projections + norms + rope + transposes + sketch ----
        with tc.tile_pool(name="proj", bufs=1) as pr, \
             tc.tile_pool(name="projs", bufs=3) as prs, \
             tc.tile_pool(name="projps", bufs=2, space="PSUM") as pps, \
             tc.tile_pool(name="trps", bufs=2, space="PSUM") as tps_pool, \
             tc.tile_pool(name="skps", bufs=1, space="PSUM") as skps:
            wq_bf = pr.tile([P, NHT * 512], BF16)
            for ht in range(NHT):
                wqf = prs.tile([P, 512], F32, tag="wload")
                nc.sync.dma_start(out=wqf[:],
                                  in_=wqkv[P * ht : P * (ht + 1), :])
                nc.any.tensor_copy(wq_bf[:, 512 * ht : 512 * (ht + 1)], wqf[:])

            ksk_ps = skps.tile([P, TSK], F32)
            vsk_ps = skps.tile([TSK, P], F32)

            for j in range(NST):
                pp = pps.tile([P, 512], F32)
                for ht in range(NHT):
                    nc.tensor.matmul(pp[:], lhsT=xt[:, S * ht + P * j : S * ht + P * (j + 1)],
                                     rhs=wq_bf[:, 512 * ht : 512 * (ht + 1)],
                                     start=(ht == 0), stop=(ht == NHT - 1))
                # rms norm of q0,q1,k (groups of 128)
                sqt = prs.tile([P, 384], F32, tag="sqt")
                nc.scalar.activation(sqt[:], pp[:, 0:384], AF.Square)
                ssq = prs.tile([P, 3], F32, tag="ssq")
                nc.vector.tensor_reduce(ssq[:], sqt[:].rearrange("p (g d) -> p g d", d=P),
                                        axis=AX.X, op=OP.add)
                nc.vector.tensor_scalar(ssq[:], ssq[:], 1.0 / HD, EPS,
                                        op0=OP.mult, op1=OP.add)
                rcp = prs.tile([P, 3], F32, tag="rcp")
                nc.vector.reciprocal(rcp[:], ssq[:])
                rno = prs.tile([P, 3], F32, tag="rno")
                nc.scalar.activation(rno[:], rcp[:], AF.Sqrt)
                qkn = prs.tile([P, 384], BF16, tag="qkn")
                for g in range(3):
                    nc.vector.tensor_scalar(qkn[:, P * g : P * (g + 1)],
                                            pp[:, P * g : P * (g + 1)],
                                            rno[:, g : g + 1], None, op0=OP.mult)
                nc.vector.tensor_tensor(qkn[:], qkn[:], qknw_sb[:], op=OP.mult)
                # v
                nc.any.tensor_copy(vsb[:, P * j : P * (j + 1)], pp[:, 384:512])
                # rope
                cosx = prs.tile([P, P], F32, tag="cosx")
                sinx = prs.tile([P, P], F32, tag="sinx")
                nc.sync.dma_start(out=cosx[:, 0:64], in_=cosj[:, 64 * j : 64 * (j + 1)])
                nc.any.tensor_copy(cosx[:, 64:P], cosx[:, 0:64])
                nc.sync.dma_start(out=sinx[:, 0:64], in_=sinj[:, 64 * j : 64 * (j + 1)])
                nc.any.tensor_copy(sinx[:, 64:P], sinx[:, 0:64])
                rot = prs.tile([P, 384], BF16, tag="rot")
                r3 = rot[:].rearrange("p (g d) -> p g d", d=P)
                q3 = qkn[:].rearrange("p (g d) -> p g d", d=P)
                nc.vector.tensor_scalar(r3[:, :, 0:64], q3[:, :, 64:P], -1.0, None,
                                        op0=OP.mult)
                nc.any.tensor_copy(r3[:, :, 64:P], q3[:, :, 0:64])
                qkr = prs.tile([P, 384], BF16, tag="qkr")
                cb3 = cosx[:].rearrange("p (o d) -> p o d", o=1).to_broadcast((P, 3, P))
                sb3 = sinx[:].rearrange("p (o d) -> p o d", o=1).to_broadcast((P, 3, P))
                nc.vector.tensor_tensor(qkr[:].rearrange("p (g d) -> p g d", d=P),
                                        q3, cb3, op=OP.mult)
                nc.vector.tensor_tensor(r3, r3, sb3, op=OP.mult)
                nc.vector.tensor_tensor(qkr[:], qkr[:], rot[:], op=OP.add)
                # transposes into [d, s] layouts
                for g, dst in ((0, qt[:, 0 * S + P * j : 0 * S + P * (j + 1)]),
                               (1, qt[:, 1 * S + P * j : 1 * S + P * (j + 1)]),
                               (2, kt[:, P * j : P * (j + 1)])):
                    tp2 = tps_pool.tile([P, P], BF16)
                    nc.tensor.transpose(tp2[:], qkr[:, P * g : P * (g + 1)], ident[:])
                    nc.any.tensor_copy(dst, tp2[:])
                # sketch accumulation (k without rope: qkn group 2; v: vsb tile)
                bt = prs.tile([P, TSK], F32, tag="bt")
                nc.sync.dma_start(out=bt[:], in_=bj[:, TSK * j : TSK * (j + 1)])
                bw = prs.tile([P, TSK], BF16, tag="bw")
                nc.vector.tensor_scalar(bw[:], bt[:], restw_pm[:, j : j + 1], None,
                                        op0=OP.mult)
                nc.tensor.matmul(ksk_ps[:], lhsT=qkn[:, 256:384], rhs=bw[:],
                                 start=(j == 0), stop=(j == NST - 1))
                nc.tensor.matmul(vsk_ps[:], lhsT=bw[:], rhs=vsb[:, P * j : P * (j + 1)],
                                 start=(j == 0), stop=(j == NST - 1))

            nc.vector.tensor_scalar(kskT_sb[:], ksk_ps[:], sksc_sb[:, 0:1], None,
                                    op0=OP.mult)
            nc.vector.tensor_scalar(vsk_sb[:], vsk_ps[:], sksc_sb[0:TSK, 0:1], None,
                                    op0=OP.mult)

        xtpool_cm.__exit__(None, None, None)

        if debug:
            nc.sync.dma_start(out=dbg["d_ksk"][:], in_=kskT_sb[:])
            nc.sync.dma_start(out=dbg["d_vsk"][:], in_=vsk_sb[:])
            nc.sync.dma_start(out=dbg["d_qt"][:], in_=qt[:, 0:S])
            nc.sync.dma_start(out=dbg["d_kt"][:], in_=kt[:])
            nc.sync.dma_start(out=dbg["d_v"][:], in_=vsb[:])

        # ---- stage 3: attention ----
        late = es.enter_context(tc.tile_pool(name="late", bufs=1))
        osb = late.tile([P, 2 * S], BF16)        # attn out^T per head
        tri_sb = late.tile([P, 4 * 512], BF16)
        nc.sync.dma_start(out=tri_sb[:], in_=tri[:])
        masksk_sb = late.tile([P, S], BF16)
        nc.sync.dma_start(out=masksk_sb[:], in_=mask_sk[:])
        with tc.tile_pool(name="att", bufs=4) as ap, \
             tc.tile_pool(name="stps", bufs=2, space="PSUM") as stp, \
             tc.tile_pool(name="dups", bufs=2, space="PSUM") as dup:
            for h in range(2):
                for b in range(NSB):
                    rhs_q = qt[:, S * h + 512 * b : S * h + 512 * (b + 1)]
                    den_ps = dup.tile([P, 512], F32, tag="den")
                    u_ps = dup.tile([P, 512], F32, tag="u")
                    ndet = 4 * (b + 1)
                    for ti in range(ndet + 1):
                        st = stp.tile([P, 512], F32)
                        ex = ap.tile([P, 512], BF16, tag="ex")
                        first = ti == 0
                        last = ti == ndet
                        if ti < ndet:
                            nc.tensor.matmul(st[:], lhsT=kt[:, P * ti : P * (ti + 1)],
                                             rhs=rhs_q, start=True, stop=True)
                            nc.scalar.activation(ex[:], st[:], AF.Exp,
                                                 bias=selbias_pm[:, ti : ti + 1],
                                                 scale=ISQ)
                            r = ti - 4 * b
                            if r >= 0:
                                nc.vector.tensor_tensor(
                                    ex[:], ex[:], tri_sb[:, 512 * r : 512 * (r + 1)],
                                    op=OP.mult)
                            nc.tensor.matmul(den_ps[:], lhsT=ones128_sb[:], rhs=ex[:],
                                             start=first, stop=False)
                            nc.tensor.matmul(u_ps[:], lhsT=vsb[:, P * ti : P * (ti + 1)],
                                             rhs=ex[:], start=first, stop=False)
                        else:
                            nc.tensor.matmul(st[0:TSK, :], lhsT=kskT_sb[:], rhs=rhs_q,
                                             start=True, stop=True)
                            nc.scalar.activation(ex[0:TSK, :], st[0:TSK, :], AF.Exp,
                                                 scale=ISQ)
                            nc.vector.tensor_tensor(
                                ex[0:TSK, :], ex[0:TSK, :],
                                masksk_sb[0:TSK, 512 * b : 512 * (b + 1)], op=OP.mult)
                            nc.tensor.matmul(den_ps[:], lhsT=ones128_sb[0:TSK, :],
                                             rhs=ex[0:TSK, :], start=False, stop=True)
                            nc.tensor.matmul(u_ps[:], lhsT=vsk_sb[:], rhs=ex[0:TSK, :],
                                             start=False, stop=True)
                    rec = ap.tile([P, 512], F32, tag="rec")
                    nc.vector.reciprocal(rec[:], den_ps[:])
                    nc.vector.tensor_tensor(osb[:, S * h + 512 * b : S * h + 512 * (b + 1)],
                                            u_ps[:], rec[:], op=OP.mult)

        if debug:
            nc.sync.dma_start(out=dbg["d_o"][:], in_=osb[:])

        # ---- stage 4: Wo partials ----
        with tc.tile_pool(name="wos", bufs=3) as wp_pool, \
             tc.tile_pool(name="wops", bufs=2, space="PSUM") as wops:
            wo_bf0 = late.tile([P, HID], BF16)
            wo_bf1 = late.tile([P, HID], BF16)
            for i, wob in enumerate([wo_bf0, wo_bf1]):
                wof = wp_pool.tile([P, HID], F32, tag="wload2")
                nc.sync.dma_start(out=wof[:], in_=wo[P * i : P * (i + 1), :])
                nc.any.tensor_copy(wob[:], wof[:])
            for m in range(NHT):
                for b2 in range(NSB):
                    wp = wops.tile([P, 512], F32)
                    nc.tensor.matmul(wp[:], lhsT=wo_bf0[:, P * m : P * (m + 1)],
                                     rhs=osb[:, 512 * b2 : 512 * (b2 + 1)],
                                     start=True, stop=False)
                    nc.tensor.matmul(wp[:], lhsT=wo_bf1[:, P * m : P * (m + 1)],
                                     rhs=osb[:, S + 512 * b2 : S + 512 * (b2 + 1)],
                                     start=False, stop=True)
                    oc = wp_pool.tile([P, 512], F32)
                    nc.any.tensor_copy(oc[:], wp[:])
                    nc.sync.dma_start(
                        out=ypart[P * m : P * (m + 1), 512 * b2 : 512 * (b2 + 1)],
                        in_=oc[:])

    _split_sync_waits(nc)
    return nc


def host_inputs(inputs):
    """Build per-core input maps from full inputs."""
    x = np.asarray(inputs["hidden_states"], np.float32)[0]            # [S, HID]
    pos = np.asarray(inputs["position_ids"])[0].astype(np.float64)    # [S]
    Wq = np.asarray(inputs["Wq"], np.float32)
    Wk = np.asarray(inputs["Wk"], np.float32)
    Wv = np.asarray(inputs["Wv"], np.float32)
    Wo = np.asarray(inputs["Wo"], np.float32)
    qnw = np.asarray(inputs["qnorm_w"], np.float32)
    knw = np.asarray(inputs["knorm_w"], np.float32)
    ka = np.asarray(inputs["kron_a"], np.float32)
    kb = np.asarray(inputs["kron_b"], np.float32)
    ssc = np.asarray(inputs["sketch_scale"], np.float32)
    W1 = np.asarray(inputs["W1"], np.float32)
    b1 = np.asarray(inputs["b1"], np.float32)
    W2 = np.asarray(inputs["W2"], np.float32)
    b2 = np.asarray(inputs["b2"], np.float32)

    half = HD // 2
    inv_freq = 1.0 / (10000.0 ** (np.arange(half, dtype=np.float64) / half))
    ang = pos[:, None] * inv_freq[None, :]
    cos_sd = np.cos(ang).astype(np.float32)      # [S, 64]
    sin_sd = np.sin(ang).astype(np.float32)
    cosj = cos_sd.reshape(NST, P, 64).transpose(1, 0, 2).reshape(P, NST * 64)
    sinj = sin_sd.reshape(NST, P, 64).transpose(1, 0, 2).reshape(P, NST * 64)

    def causal_mask(rows, cols):
        m = np.zeros((rows, cols), np.float32)
        r = cols / rows
        for i in range(rows):
            m[i, : int((i + 1) * r)] = 1.0
        return m

    ca = ka * causal_mask(KA_R, KA_C)            # [20, 128]
    cb = kb * causal_mask(KB_R, KB_C)            # [32, 256]
    # B[s, t] with s = p*256+q (p<16), t = c*20+a (c<4)
    Bmat = np.einsum("cq,ap->pqca", cb[:4], ca[:, :16]).reshape(S, TSK).astype(np.float32)
    bjarr = Bmat.reshape(NST, P, TSK).transpose(1, 0, 2).reshape(P, NST * TSK)

    ratio = (KA_C * KB_C) / SKETCH
    sk_times = (np.arange(SKETCH) * ratio).astype(np.float32)[:TSK]
    msk = (sk_times[:, None] <= np.arange(S)[None, :]).astype(np.float32)  # [80, S]
    mask_sk = np.zeros((P, S), np.float32)
    mask_sk[:TSK] = msk

    tri = np.zeros((P, 4 * 512), np.float32)
    for r in range(4):
        tp_idx = np.arange(P)[:, None]
        q_idx = np.arange(512)[None, :]
        tri[:, 512 * r : 512 * (r + 1)] = (P * r + tp_idx <= q_idx)

    b2c = (b2.reshape(1, 1) - math.log(S / SKETCH)).astype(np.float32)
    qknormw = np.concatenate([qnw, qnw, knw]).reshape(1, 384).astype(np.float32)

    common = dict(
        x=np.ascontiguousarray(x),
        w1=np.ascontiguousarray(W1),
        w2=np.ascontiguousarray(W2),
        b1c=np.ascontiguousarray(b1.reshape(64, 1)),
        b2c=b2c,
        qknormw=qknormw,
        skscale=ssc.reshape(1, 1),
        cosj=cosj, sinj=sinj, bj=bjarr,
        mask_sk=mask_sk.astype(np.float32),
        tri=tri,
        iota1=(np.arange(1, P + 1, dtype=np.float32)).reshape(P, 1),
        onescol=np.ones((P, 1), np.float32),
        ones1r=np.ones((1, P), np.float32),
        ones128=np.ones((P, P), np.float32),
    )
    in_maps = []
    for c in range(NCORES):
        g = c // 2
        wqkv = np.concatenate(
            [Wq[:, 256 * c : 256 * (c + 1)],
             Wk[:, 128 * g : 128 * (g + 1)],
             Wv[:, 128 * g : 128 * (g + 1)]], axis=1)
        m = dict(common)
        m["wqkv"] = np.ascontiguousarray(wqkv)
        m["wo"] = np.ascontiguousarray(Wo[256 * c : 256 * (c + 1), :])
        in_maps.append(m)
    return in_maps


_cache = {}


def _get_program(debug=False):
    key = ("nc", debug)
    if key not in _cache:
        _cache[key] = build_program(debug=debug)
    return _cache[key]


def _get_runner(debug=False):
    """Build (once) a cached jitted shard_map executor for the program,
    mirroring bass2jax.run_bass_via_pjrt but reusable across calls."""
    key = ("runner", debug)
    if key in _cache:
        return _cache[key]
    import jax
    from jax.sharding import Mesh, PartitionSpec
    from jax.experimental.shard_map import shard_map
    from concourse import bass2jax

    nc = _get_program(debug=debug)
    bass2jax.install_neuronx_cc_hook()

    partition_name = nc.partition_id_tensor.name if nc.partition_id_tensor else None
    in_names, out_names, out_avals, zero_outs = [], [], [], []
    for alloc in nc.m.functions[0].allocations:
        if not isinstance(alloc, mybir.MemoryLocationSet):
            continue
        name = alloc.memorylocations[0].name
        if alloc.kind == "ExternalInput":
            if name != partition_name:
                in_names.append(name)
        elif alloc.kind == "ExternalOutput":
            out_names.append(name)
            shape = tuple(alloc.tensor_shape)
            dtype = mybir.dt.np(alloc.dtype)
            out_avals.append(jax.core.ShapedArray(shape, dtype))
            zero_outs.append(np.zeros(shape, dtype))
    n_params = len(in_names)
    n_outs = len(out_avals)
    all_names = in_names + out_names
    if partition_name is not None:
        all_names = all_names + [partition_name]
    donate = tuple(range(n_params, n_params + n_outs))

    def _body(*args):
        operands = list(args)
        if partition_name is not None:
            operands.append(bass2jax.partition_id_tensor())
        outs = bass2jax._bass_exec_p.bind(
            *operands,
            out_avals=tuple(out_avals),
            in_names=tuple(all_names),
            out_names=tuple(out_names),
            lowering_input_output_aliases=(),
            sim_require_finite=True,
            sim_require_nnan=True,
            nc=nc,
        )
        return tuple(outs)

    devices = jax.devices()[:NCORES]
    mesh = Mesh(np.asarray(devices), ("core",))
    sharded = jax.jit(
        shard_map(_body, mesh=mesh,
                  in_specs=(PartitionSpec("core"),) * (n_params + n_outs),
                  out_specs=(PartitionSpec("core"),) * n_outs,
                  check_rep=False),
        donate_argnums=donate, keep_unused=True)

    def execute(in_maps):
        concat_in = [
            np.concatenate([np.asarray(in_maps[c][nm]) for c in range(NCORES)], axis=0)
            for nm in in_names
        ]
        concat_zeros = [
            np.zeros((NCORES * z.shape[0], *z.shape[1:]), z.dtype) for z in zero_outs
        ]
        out_arrs = sharded(*concat_in, *concat_zeros)
        jax.block_until_ready(out_arrs)
        return [
            {nm: np.asarray(out_arrs[i]).reshape(NCORES, *out_avals[i].shape)[c]
             for i, nm in enumerate(out_names)}
            for c in range(NCORES)
        ]

    _cache[key] = execute
    return execute


class _Res:
    def __init__(self, results):
        self.results = results


def run(inputs, debug=False, trace=False):
    import ml_dtypes
    in_maps = host_inputs(inputs)
    for m in in_maps:
        for k in ("mask_sk", "tri", "ones128"):
            m[k] = m[k].astype(ml_dtypes.bfloat16)
    execute = _get_runner(debug=debug)
    return _Res(execute(in_maps)), in_maps


def kernel(**inputs) -> np.ndarray:
    res, _ = run(inputs)
    y = np.zeros((HID, S), np.float64)
    for c in range(NCORES):
        y += res.results[c]["ypart"].astype(np.float64)
    return y.T.reshape(B, S, HID).astype(np.float32)


# revision 70
# speedup vs baseline: 36022.4722x; 36022.4722x over previous
"""Trainium2 Bass kernel for nn_RandNLAGQALayer (sparse attention, GQA, kron-sketch).

Sharding: 8-way tensor-parallel over heads. Each core computes 2 q-heads and its
(duplicated across a pair) kv head, full sequence. Importance scorer + top-k
threshold + sketch are computed per-core (replicated, tiny). Output projection
partial sums are reduced on host.
"""

import math
import numpy as np

import concourse.bass as bass
import concourse.mybir as mybir
import concourse.tile as tile
from concourse.tile import TileContext, ScopedClock
from concourse.masks import make_identity
from concourse.bass_utils import run_bass_kernel_spmd

F32 = mybir.dt.float32
BF16 = mybir.dt.bfloat16
AX = mybir.AxisListType
OP = mybir.AluOpType
AF = mybir.ActivationFunctionType

B, S, HID = 1, 4096, 2048
NH, NKV, HD = 16, 4, 128
SKETCH, TOPK = 640, 2048
KA_R, KA_C = 20, 128
KB_R, KB_C = 32, 256
EPS = 1e-6
NCORES = 8
P = 128
NST = S // P            # 32 s-tiles
NHT = HID // P          # 16 h-tiles
NSB = S // 512          # 8 sq blocks
TSK = 80                # sketch keys that can ever be unmasked (t*51.2<=4095)
ISQ = 1.0 / math.sqrt(HD)

def _split_sync_waits(nc, maxw=1):
    """This container's walrus rejects instructions carrying more than ~1 sync
    wait. Move extra waits onto same-engine nops inserted just before."""
    for f in nc.m.functions:
        for bb in f.blocks:
            newlist = []
            for ins in bb.instructions:
                si = ins.sync_info
                waits = list(si.on_wait) if si and si.on_wait else []
                if len(waits) > maxw:
                    si.on_wait = waits[:maxw]
                    for w in waits[maxw:]:
                        nop = mybir.InstNoOp(name=f"I-{nc.next_id()}", ins=[], outs=[])
                        nop.engine = ins.engine
                        nop.sync_info = mybir.SyncInfo(on_wait=[w], on_update=[])
                        nc.register_instruction(nop, overwrite=True)
                        newlist.append(nop)
                newlist.append(ins)
            bb.instructions[:] = newlist


def build_program(debug=False):
    nc = bass.Bass("TRN2", target_bir_lowering=False, debug=False, num_devices=NCORES)

    def din(name, shape):
        return nc.dram_tensor(name, shape, F32, kind="ExternalInput")

    xth = nc.dram_tensor("xth", [HID, S], BF16, kind="ExternalInput")
    xtl = nc.dram_tensor("xtl", [HID, S], BF16, kind="ExternalInput")
    wqkv = nc.dram_tensor("wqkv", [HID, 512], BF16, kind="ExternalInput")
    wo = nc.dram_tensor("wo", [256, HID], BF16, kind="ExternalInput")
    w1b_in = nc.dram_tensor("w1b_in", [P, NHT * 64], BF16, kind="ExternalInput")
    w1lo_in = nc.dram_tensor("w1lo_in", [P, NHT * 64], BF16, kind="ExternalInput")
    w2 = din("w2", [64, 1])
    b1c = din("b1c", [64, 1])
    b2c = din("b2c", [1, 1])
    qknormw = din("qknormw", [1, 384])
    skscale = din("skscale", [1, 1])
    cosj = din("cosj", [P, NST * 64])
    sinj = din("sinj", [P, NST * 64])
    bj = din("bj", [P, NST * TSK])
    mask_sk = nc.dram_tensor("mask_sk", [P, S], BF16, kind="ExternalInput")
    tri = nc.dram_tensor("tri", [P, 8 * 512], BF16, kind="ExternalInput")
    iota1 = din("iota1", [P, 1])
    onescol = din("onescol", [P, 1])
    ones1r = din("ones1r", [1, P])
    ones128 = nc.dram_tensor("ones128", [P, P], BF16, kind="ExternalInput")

    ypart = nc.dram_tensor("ypart", [HID, S], BF16, kind="ExternalOutput")
    ldram = nc.dram_tensor("ldram", [1, S], F32)

    dbg = {}
    if debug:
        for nm, shp in [
            ("d_logits", [P, NST]), ("d_tau", [P, 1]), ("d_restw", [P, NST]),
            ("d_selb", [P, NST]),
        ]:
            dbg[nm] = nc.dram_tensor(nm, shp, F32, kind="ExternalOutput")
        for nm, shp in [
            ("d_ksk", [P, TSK]), ("d_vsk", [TSK, P]),
            ("d_qt", [P, S]), ("d_kt", [P, S]), ("d_v", [P, S]),
            ("d_o", [P, 2 * S]),
        ]:
            dbg[nm] = nc.dram_tensor(nm, shp, BF16, kind="ExternalOutput")

    from contextlib import ExitStack
    with TileContext(nc) as tc, ExitStack() as es:
        cpool = es.enter_context(tc.tile_pool(name="consts", bufs=1))
        big = es.enter_context(tc.tile_pool(name="big", bufs=1))

        ident = cpool.tile([P, P], BF16)
        make_identity(nc, ident[:])
        iota1_sb = cpool.tile([P, 1], F32)
        nc.sync.dma_start(out=iota1_sb[:], in_=iota1[:])
        onescol_sb = cpool.tile([P, 1], F32)
        nc.sync.dma_start(out=onescol_sb[:], in_=onescol[:])
        ones1r_sb = cpool.tile([1, P], F32)
        nc.sync.dma_start(out=ones1r_sb[:], in_=ones1r[:])
        ones128_sb = cpool.tile([P, P], BF16)
        nc.sync.dma_start(out=ones128_sb[:], in_=ones128[:])
        qknw_sb = cpool.tile([P, 384], F32)
        nc.sync.dma_start(out=qknw_sb[:], in_=qknormw[:].to_broadcast((P, 384)))
        b2c_sb = cpool.tile([P, 1], F32)
        nc.sync.dma_start(out=b2c_sb[:], in_=b2c[:].to_broadcast((P, 1)))
        sksc_sb = cpool.tile([P, 1], F32)
        nc.sync.dma_start(out=sksc_sb[:], in_=skscale[:].to_broadcast((P, 1)))
        b1c_sb = cpool.tile([64, 1], F32)
        nc.sync.dma_start(out=b1c_sb[:], in_=b1c[:])

        # persistent big tensors
        qt = big.tile([P, 2 * S], BF16)          # q^T roped, head h at [S*h : ...]
        kt = big.tile([P, S], BF16)              # k^T roped
        vsb = big.tile([P, S], BF16)             # v in [s,d], s-tile j at cols [128j:...]
        wq_bf = big.tile([P, NHT * 512], BF16)
        lg_stuff = es.enter_context(tc.tile_pool(name="lgp", bufs=1))
        logits_pm = lg_stuff.tile([P, NST], F32)
        selbias_pm = lg_stuff.tile([P, NST], F32)
        restw_pm = lg_stuff.tile([P, NST], F32)
        sk_stuff = es.enter_context(tc.tile_pool(name="skp", bufs=1))
        kskT_sb = sk_stuff.tile([P, TSK], BF16)
        vsk_sb = sk_stuff.tile([TSK, P], BF16)
        xtpool_cm = tc.tile_pool(name="xtp", bufs=1)
        xtpool = xtpool_cm.__enter__()
        xt = xtpool.tile([P, NHT * S], BF16)     # x^T, h-tile ht at cols [S*ht : S*(ht+1)]

        # ---- stage 1: weights + logits + threshold + rest_w ----
        with tc.tile_pool(name="lgA", bufs=1) as lp, \
             tc.tile_pool(name="lgps", bufs=1, space="PSUM") as lps:
            w1b = lp.tile([P, NHT * 64], BF16)
            nc.sync.dma_start(out=w1b[:], in_=w1b_in[:])
            w1lo = lp.tile([P, NHT * 64], BF16)
            nc.sync.dma_start(out=w1lo[:], in_=w1lo_in[:])
            w2f = lp.tile([64, 1], F32)
            nc.sync.dma_start(out=w2f[:], in_=w2[:])
            for ht in range(NHT):
                nc.sync.dma_start(out=wq_bf[:, 512 * ht : 512 * (ht + 1)],
                                  in_=wqkv[P * ht : P * (ht + 1), :])

            # u = x@W1 in split-bf16 (hi*hi + lo*hi + hi*lo); the top-k
            # threshold needs logits accurate to ~1e-5. Interleave the bulk
            # bf16 xt load with the xlo stream.
            lg_ps = lps.tile([P, NST], F32)
            with tc.tile_pool(name="xlos", bufs=2) as xlp, \
                 tc.tile_pool(name="ttp", bufs=2) as ttp:
                for g in range(2):
                    gps = []
                    for i in range(4):
                        lg_u_ps = lps.tile([64, 512], F32, tag=f"lgu{i}")
                        gps.append(lg_u_ps)
                    for ht in range(NHT):
                        xlt = xlp.tile([P, 2048], BF16)
                        nc.sync.dma_start(
                            out=xlt[:],
                            in_=xtl[P * ht : P * (ht + 1),
                                    2048 * g : 2048 * (g + 1)])
                        if g == 0:
                            nc.sync.dma_start(out=xt[:, S * ht : S * (ht + 1)],
                                              in_=xth[P * ht : P * (ht + 1), :])
                        w1b_t = w1b[:, 64 * ht : 64 * (ht + 1)]
                        w1lo_t = w1lo[:, 64 * ht : 64 * (ht + 1)]
                        for i in range(4):
                            sl = slice(S * ht + 2048 * g + 512 * i,
                                       S * ht + 2048 * g + 512 * (i + 1))
                            nc.tensor.matmul(gps[i][:], lhsT=w1b_t, rhs=xt[:, sl],
                                             start=(ht == 0), stop=False)
                            nc.tensor.matmul(gps[i][:], lhsT=w1lo_t, rhs=xt[:, sl],
                                             start=False, stop=False)
                            nc.tensor.matmul(gps[i][:], lhsT=w1b_t,
                                             rhs=xlt[:, 512 * i : 512 * (i + 1)],
                                             start=False, stop=(ht == NHT - 1))
                    for i in range(4):
                        sb = 4 * g + i
                        tt_t = ttp.tile([64, 512], F32, tag="tt")
                        nc.scalar.activation(tt_t[:], gps[i][:], AF.Tanh,
                                             bias=b1c_sb[:])
                        for jj in range(4):
                            j = 4 * sb + jj
                            nc.tensor.matmul(lg_ps[:, j : j + 1],
                                             lhsT=tt_t[:, P * jj : P * (jj + 1)],
                                             rhs=w2f[:], start=True, stop=True)
            nc.vector.tensor_scalar(logits_pm[:], lg_ps[:], b2c_sb[:, 0:1], None,
                                    op0=OP.add)

        with tc.tile_pool(name="lgB", bufs=1) as lp, \
             tc.tile_pool(name="lgBps", bufs=2, space="PSUM") as lps:
            # threshold search
            nc.sync.dma_start(out=ldram[0:1, :].rearrange("o (j p) -> o p j", p=P),
                              in_=logits_pm[:])
            lrep = lp.tile([P, S], F32)
            nc.sync.dma_start(out=lrep[:], in_=ldram[:].to_broadcast((P, S)))
            lo = lp.tile([P, 1], F32)
            wd = lp.tile([P, 1], F32)
            nc.vector.memset(lo[:], -6.0)
            nc.vector.memset(wd[:], 8.0 / 128.0)
            cmp = lp.tile([P, S], BF16)
            thr = lp.tile([P, 1], F32)
            cnt = lp.tile([P, 1], F32)
            ge = lp.tile([P, 1], F32)
            nselb = lp.tile([P, 1], F32)
            nsel_sb = lp.tile([1, 1], F32)
            NLVL = 3
            for lvl in range(NLVL):
                nc.vector.tensor_tensor(thr[:], iota1_sb[:], wd[:], op=OP.mult)
                nc.vector.tensor_tensor(thr[:], thr[:], lo[:], op=OP.add)
                nc.vector.tensor_scalar(cmp[:], lrep[:], thr[:, 0:1], 0.0,
                                        op0=OP.is_gt, op1=OP.add, accum_out=cnt[:])
                nc.vector.tensor_scalar(ge[:], cnt[:], 2047.5, None, op0=OP.is_ge)
                ns_ps = lps.tile([1, 1], F32)
                nc.tensor.matmul(ns_ps[:], lhsT=onescol_sb[:], rhs=ge[:],
                                 start=True, stop=True)
                nc.any.tensor_copy(nsel_sb[:], ns_ps[:])
                nsb_ps = lps.tile([P, 1], F32)
                nc.tensor.matmul(nsb_ps[:], lhsT=ones1r_sb[:], rhs=nsel_sb[:],
                                 start=True, stop=True)
                nc.any.tensor_copy(nselb[:], nsb_ps[:])
                nc.vector.tensor_tensor(nselb[:], nselb[:], wd[:], op=OP.mult)
                nc.vector.tensor_tensor(lo[:], lo[:], nselb[:], op=OP.add)
                if lvl < NLVL - 1:
                    nc.vector.tensor_scalar(wd[:], wd[:], 1.0 / 128.0, None, op0=OP.mult)

            nc.vector.tensor_scalar(selbias_pm[:], logits_pm[:], lo[:, 0:1], -1e4,
                                    op0=OP.is_le, op1=OP.mult)
            iw_pm = lp.tile([P, NST], F32)
            nc.scalar.activation(iw_pm[:], logits_pm[:], AF.Sigmoid)
            nc.vector.tensor_scalar(restw_pm[:], logits_pm[:], lo[:, 0:1], None,
                                    op0=OP.is_le)
            nc.vector.tensor_tensor(restw_pm[:], restw_pm[:], iw_pm[:], op=OP.mult)
            if debug:
                nc.sync.dma_start(out=dbg["d_logits"][:], in_=logits_pm[:])
                dtau = lp.tile([P, 1], F32)
                nc.any.tensor_copy(dtau[:], lo[:])
                nc.sync.dma_start(out=dbg["d_tau"][:], in_=dtau[:])
                nc.sync.dma_start(out=dbg["d_restw"][:], in_=restw_pm[:])
                nc.sync.dma_start(out=dbg["d_selb"][:], in_=selbias_pm[:])

        # ---- stage 2: projections + norms + rope + transposes + sketch ----
        with tc.tile_pool(name="proj", bufs=1) as pr, \
             tc.tile_pool(name="projs", bufs=3) as prs, \
             tc.tile_pool(name="projps", bufs=2, space="PSUM") as pps, \
             tc.tile_pool(name="trps", bufs=2, space="PSUM") as tps_pool, \
             tc.tile_pool(name="skps", bufs=1, space="PSUM") as skps:
            ksk_ps = skps.tile([P, TSK], F32)
            vsk_ps = skps.tile([TSK, P], F32)

            for j in range(NST):
                pp = pps.tile([P, 512], F32)
                for ht in range(NHT):
                    nc.tensor.matmul(pp[:], lhsT=xt[:, S * ht + P * j : S * ht + P * (j + 1)],
                                     rhs=wq_bf[:, 512 * ht : 512 * (ht + 1)],
                                     start=(ht == 0), stop=(ht == NHT - 1))
                # rms norm of q0,q1,k (groups of 128)
                sqt = prs.tile([P, 384], F32, tag="sqt")
                nc.scalar.activation(sqt[:], pp[:, 0:384], AF.Square)
                ssq = prs.tile([P, 3], F32, tag="ssq")
                nc.vector.tensor_reduce(ssq[:], sqt[:].rearrange("p (g d) -> p g d", d=P),
                                        axis=AX.X, op=OP.add)
                nc.vector.tensor_scalar(ssq[:], ssq[:], 1.0 / HD, EPS,
                                        op0=OP.mult, op1=OP.add)
                rcp = prs.tile([P, 3], F32, tag="rcp")
                nc.vector.reciprocal(rcp[:], ssq[:])
                rno = prs.tile([P, 3], F32, tag="rno")
                nc.scalar.activation(rno[:], rcp[:], AF.Sqrt)
                qkn = prs.tile([P, 384], BF16, tag="qkn")
                for g in range(3):
                    nc.vector.tensor_scalar(qkn[:, P * g : P * (g + 1)],
                                            pp[:, P * g : P * (g + 1)],
                                            rno[:, g : g + 1], None, op0=OP.mult)
                nc.vector.tensor_tensor(qkn[:], qkn[:], qknw_sb[:], op=OP.mult)
                # v
                nc.any.tensor_copy(vsb[:, P * j : P * (j + 1)], pp[:, 384:512])
                # rope
                cosx = prs.tile([P, P], F32, tag="cosx")
                sinx = prs.tile([P, P], F32, tag="sinx")
                nc.sync.dma_start(out=cosx[:, 0:64], in_=cosj[:, 64 * j : 64 * (j + 1)])
                nc.any.tensor_copy(cosx[:, 64:P], cosx[:, 0:64])
                nc.sync.dma_start(out=sinx[:, 0:64], in_=sinj[:, 64 * j : 64 * (j + 1)])
                nc.any.tensor_copy(sinx[:, 64:P], sinx[:, 0:64])
                rot = prs.tile([P, 384], BF16, tag="rot")
                r3 = rot[:].rearrange("p (g d) -> p g d", d=P)
                q3 = qkn[:].rearrange("p (g d) -> p g d", d=P)
                nc.vector.tensor_scalar(r3[:, :, 0:64], q3[:, :, 64:P], -1.0, None,
                                        op0=OP.mult)
                nc.any.tensor_copy(r3[:, :, 64:P], q3[:, :, 0:64])
                qkr = prs.tile([P, 384], BF16, tag="qkr")
                cb3 = cosx[:].rearrange("p (o d) -> p o d", o=1).to_broadcast((P, 3, P))
                sb3 = sinx[:].rearrange("p (o d) -> p o d", o=1).to_broadcast((P, 3, P))
                nc.vector.tensor_tensor(qkr[:].rearrange("p (g d) -> p g d", d=P),
                                        q3, cb3, op=OP.mult)
                nc.vector.tensor_tensor(r3, r3, sb3, op=OP.mult)
                nc.vector.tensor_tensor(qkr[:], qkr[:], rot[:], op=OP.add)
                # transposes into [d, s] layouts
                for g, dst in ((0, qt[:, 0 * S + P * j : 0 * S + P * (j + 1)]),
                               (1, qt[:, 1 * S + P * j : 1 * S + P * (j + 1)]),
                               (2, kt[:, P * j : P * (j + 1)])):
                    tp2 = tps_pool.tile([P, P], BF16)
                    nc.tensor.transpose(tp2[:], qkr[:, P * g : P * (g + 1)], ident[:])
                    nc.any.tensor_copy(dst, tp2[:])
                # sketch accumulation (k without rope: qkn group 2; v: vsb tile)
                bt = prs.tile([P, TSK], F32, tag="bt")
                nc.sync.dma_start(out=bt[:], in_=bj[:, TSK * j : TSK * (j + 1)])
                bw = prs.tile([P, TSK], BF16, tag="bw")
                nc.vector.tensor_scalar(bw[:], bt[:], restw_pm[:, j : j + 1], None,
                                        op0=OP.mult)
                nc.tensor.matmul(ksk_ps[:], lhsT=qkn[:, 256:384], rhs=bw[:],
                                 start=(j == 0), stop=(j == NST - 1))
                nc.tensor.matmul(vsk_ps[:], lhsT=bw[:], rhs=vsb[:, P * j : P * (j + 1)],
                                 start=(j == 0), stop=(j == NST - 1))

            nc.vector.tensor_scalar(kskT_sb[:], ksk_ps[:], sksc_sb[:, 0:1], None,
                                    op0=OP.mult)
            nc.vector.tensor_scalar(vsk_sb[:], vsk_ps[:], sksc_sb[0:TSK, 0:1], None,
                                    op0=OP.mult)

        xtpool_cm.__exit__(None, None, None)

        if debug:
            nc.sync.dma_start(out=dbg["d_ksk"][:], in_=kskT_sb[:])
            nc.sync.dma_start(out=dbg["d_vsk"][:], in_=vsk_sb[:])
            nc.sync.dma_start(out=dbg["d_qt"][:], in_=qt[:, 0:S])
            nc.sync.dma_start(out=dbg["d_kt"][:], in_=kt[:])
            nc.sync.dma_start(out=dbg["d_v"][:], in_=vsb[:])

        # ---- stage 3: attention ----
        late = es.enter_context(tc.tile_pool(name="late", bufs=1))
        osb = late.tile([P, 2 * S], BF16)        # attn out^T per head
        tri_sb = late.tile([P, 8 * 512], BF16)
        nc.sync.dma_start(out=tri_sb[:], in_=tri[:])
        masksk_sb = late.tile([P, S], BF16)
        nc.sync.dma_start(out=masksk_sb[:], in_=mask_sk[:])
        NB = 512
        NTRI = NB // P
        with tc.tile_pool(name="att", bufs=6) as ap, \
             tc.tile_pool(name="stps", bufs=3, space="PSUM") as stp, \
             tc.tile_pool(name="dups", bufs=2, space="PSUM") as dup:
            for h in range(2):
                for b in range(S // NB):
                    rhs_q = qt[:, S * h + NB * b : S * h + NB * (b + 1)]
                    den_ps = dup.tile([P, NB], F32, tag="den")
                    u_ps = dup.tile([P, NB], F32, tag="u")
                    ndet = NTRI * (b + 1)
                    for ti in range(ndet + 1):
                        st = stp.tile([P, NB], F32)
                        ex = ap.tile([P, NB], BF16, tag="ex")
                        first = ti == 0
                        if ti < ndet:
                            nc.tensor.matmul(st[:], lhsT=kt[:, P * ti : P * (ti + 1)],
                                             rhs=rhs_q, start=True, stop=True)
                            nc.scalar.activation(ex[:], st[:], AF.Exp,
                                                 bias=selbias_pm[:, ti : ti + 1],
                                                 scale=ISQ)
                            r = ti - NTRI * b
                            if r >= 0:
                                nc.gpsimd.tensor_tensor(
                                    ex[:], ex[:], tri_sb[:, NB * r : NB * (r + 1)],
                                    op=OP.mult)
                            nc.tensor.matmul(den_ps[:], lhsT=ones128_sb[:], rhs=ex[:],
                                             start=first, stop=False)
                            nc.tensor.matmul(u_ps[:], lhsT=vsb[:, P * ti : P * (ti + 1)],
                                             rhs=ex[:], start=first, stop=False)
                        else:
                            nc.tensor.matmul(st[0:TSK, :], lhsT=kskT_sb[:], rhs=rhs_q,
                                             start=True, stop=True)
                            nc.scalar.activation(ex[0:TSK, :], st[0:TSK, :], AF.Exp,
                                                 scale=ISQ)
                            nc.gpsimd.tensor_tensor(
                                ex[0:TSK, :], ex[0:TSK, :],
                                masksk_sb[0:TSK, NB * b : NB * (b + 1)], op=OP.mult)
                            nc.tensor.matmul(den_ps[:], lhsT=ones128_sb[0:TSK, :],
                                             rhs=ex[0:TSK, :], start=False, stop=True)
                            nc.tensor.matmul(u_ps[:], lhsT=vsk_sb[:], rhs=ex[0:TSK, :],
                                             start=False, stop=True)
                    rec = ap.tile([P, NB], F32, tag="rec")
                    nc.vector.reciprocal(rec[:], den_ps[:])
                    nc.vector.tensor_tensor(osb[:, S * h + NB * b : S * h + NB * (b + 1)],
                                            u_ps[:], rec[:], op=OP.mult)

        if debug:
            nc.sync.dma_start(out=dbg["d_o"][:], in_=osb[:])

        # ---- stage 4: Wo partials ----
        with tc.tile_pool(name="wos", bufs=3) as wp_pool, \
             tc.tile_pool(name="wops", bufs=2, space="PSUM") as wops:
            wo_bf0 = late.tile([P, HID], BF16)
            wo_bf1 = late.tile([P, HID], BF16)
            for i, wob in enumerate([wo_bf0, wo_bf1]):
                nc.sync.dma_start(out=wob[:], in_=wo[P * i : P * (i + 1), :])
            for m in range(NHT):
                for bg in range(2):
                    wps = []
                    for i in range(4):
                        wp = wops.tile([P, 512], F32, tag=f"wp{i}")
                        wps.append(wp)
                    # weight-stationary: one LDW per (m, head) reused over 4 blocks
                    for hh, wob in enumerate([wo_bf0, wo_bf1]):
                        for i in range(4):
                            b2 = 4 * bg + i
                            nc.tensor.matmul(
                                wps[i][:], lhsT=wob[:, P * m : P * (m + 1)],
                                rhs=osb[:, S * hh + 512 * b2 : S * hh + 512 * (b2 + 1)],
                                start=(hh == 0), stop=(hh == 1))
                    for i in range(4):
                        b2 = 4 * bg + i
                        oc = wp_pool.tile([P, 512], BF16)
                        nc.any.tensor_copy(oc[:], wps[i][:])
                        nc.sync.dma_start(
                            out=ypart[P * m : P * (m + 1), 512 * b2 : 512 * (b2 + 1)],
                            in_=oc[:])

    _split_sync_waits(nc)
    return nc


def host_inputs(inputs):
    """Build per-core input maps from full inputs."""
    x = np.asarray(inputs["hidden_states"], np.float32)[0]            # [S, HID]
    pos = np.asarray(inputs["position_ids"])[0].astype(np.float64)    # [S]
    Wq = np.asarray(inputs["Wq"], np.float32)
    Wk = np.asarray(inputs["Wk"], np.float32)
    Wv = np.asarray(inputs["Wv"], np.float32)
    Wo = np.asarray(inputs["Wo"], np.float32)
    qnw = np.asarray(inputs["qnorm_w"], np.float32)
    knw = np.asarray(inputs["knorm_w"], np.float32)
    ka = np.asarray(inputs["kron_a"], np.float32)
    kb = np.asarray(inputs["kron_b"], np.float32)
    ssc = np.asarray(inputs["sketch_scale"], np.float32)
    W1 = np.asarray(inputs["W1"], np.float32)
    b1 = np.asarray(inputs["b1"], np.float32)
    W2 = np.asarray(inputs["W2"], np.float32)
    b2 = np.asarray(inputs["b2"], np.float32)

    half = HD // 2
    inv_freq = 1.0 / (10000.0 ** (np.arange(half, dtype=np.float64) / half))
    ang = pos[:, None] * inv_freq[None, :]
    cos_sd = np.cos(ang).astype(np.float32)      # [S, 64]
    sin_sd = np.sin(ang).astype(np.float32)
    cosj = cos_sd.reshape(NST, P, 64).transpose(1, 0, 2).reshape(P, NST * 64)
    sinj = sin_sd.reshape(NST, P, 64).transpose(1, 0, 2).reshape(P, NST * 64)

    def causal_mask(rows, cols):
        m = np.zeros((rows, cols), np.float32)
        r = cols / rows
        for i in range(rows):
            m[i, : int((i + 1) * r)] = 1.0
        return m

    ca = ka * causal_mask(KA_R, KA_C)            # [20, 128]
    cb = kb * causal_mask(KB_R, KB_C)            # [32, 256]
    # B[s, t] with s = p*256+q (p<16), t = c*20+a (c<4)
    Bmat = np.einsum("cq,ap->pqca", cb[:4], ca[:, :16]).reshape(S, TSK).astype(np.float32)
    bjarr = Bmat.reshape(NST, P, TSK).transpose(1, 0, 2).reshape(P, NST * TSK)

    ratio = (KA_C * KB_C) / SKETCH
    sk_times = (np.arange(SKETCH) * ratio).astype(np.float32)[:TSK]
    msk = (sk_times[:, None] <= np.arange(S)[None, :]).astype(np.float32)  # [80, S]
    mask_sk = np.zeros((P, S), np.float32)
    mask_sk[:TSK] = msk

    tri = np.zeros((P, 8 * 512), np.float32)
    for r in range(8):
        tp_idx = np.arange(P)[:, None]
        q_idx = np.arange(512)[None, :]
        tri[:, 512 * r : 512 * (r + 1)] = (P * r + tp_idx <= q_idx)

    b2c = (b2.reshape(1, 1) - math.log(S / SKETCH)).astype(np.float32)
    qknormw = np.concatenate([qnw, qnw, knw]).reshape(1, 384).astype(np.float32)

    import ml_dtypes
    xT = np.ascontiguousarray(x.T)                       # [HID, S] f32
    xth = xT.astype(ml_dtypes.bfloat16)
    xtl = (xT - xth.astype(np.float32)).astype(ml_dtypes.bfloat16)
    w1p = np.ascontiguousarray(
        W1.reshape(NHT, P, 64).transpose(1, 0, 2).reshape(P, NHT * 64))
    w1b_in = w1p.astype(ml_dtypes.bfloat16)
    w1lo_in = (w1p - w1b_in.astype(np.float32)).astype(ml_dtypes.bfloat16)
    common = dict(
        xth=xth, xtl=xtl, w1b_in=w1b_in, w1lo_in=w1lo_in,
        w2=np.ascontiguousarray(W2),
        b1c=np.ascontiguousarray(b1.reshape(64, 1)),
        b2c=b2c,
        qknormw=qknormw,
        skscale=ssc.reshape(1, 1),
        cosj=cosj, sinj=sinj, bj=bjarr,
        mask_sk=mask_sk.astype(np.float32),
        tri=tri,
        iota1=(np.arange(1, P + 1, dtype=np.float32)).reshape(P, 1),
        onescol=np.ones((P, 1), np.float32),
        ones1r=np.ones((1, P), np.float32),
        ones128=np.ones((P, P), np.float32),
    )
    in_maps = []
    for c in range(NCORES):
        g = c // 2
        wqkv = np.concatenate(
            [Wq[:, 256 * c : 256 * (c + 1)],
             Wk[:, 128 * g : 128 * (g + 1)],
             Wv[:, 128 * g : 128 * (g + 1)]], axis=1)
        m = dict(common)
        m["wqkv"] = np.ascontiguousarray(wqkv).astype(ml_dtypes.bfloat16)
        m["wo"] = np.ascontiguousarray(
            Wo[256 * c : 256 * (c + 1), :]).astype(ml_dtypes.bfloat16)
        in_maps.append(m)
    return in_maps


_cache = {}


def _get_program(debug=False):
    key = ("nc", debug)
    if key not in _cache:
        _cache[key] = build_program(debug=debug)
    return _cache[key]


def _get_runner(debug=False):
    """Build (once) a cached jitted shard_map executor for the program,
    mirroring bass2jax.run_bass_via_pjrt but reusable across calls."""
    key = ("runner", debug)
    if key in _cache:
        return _cache[key]
    import jax
    from jax.sharding import Mesh, PartitionSpec
    from jax.experimental.shard_map import shard_map
    from concourse import bass2jax

    nc = _get_program(debug=debug)
    bass2jax.install_neuronx_cc_hook()

    partition_name = nc.partition_id_tensor.name if nc.partition_id_tensor else None
    in_names, out_names, out_avals, zero_outs = [], [], [], []
    for alloc in nc.m.functions[0].allocations:
        if not isinstance(alloc, mybir.MemoryLocationSet):
            continue
        name = alloc.memorylocations[0].name
        if alloc.kind == "ExternalInput":
            if name != partition_name:
                in_names.append(name)
        elif alloc.kind == "ExternalOutput":
            out_names.append(name)
            shape = tuple(alloc.tensor_shape)
            dtype = mybir.dt.np(alloc.dtype)
            out_avals.append(jax.core.ShapedArray(shape, dtype))
            zero_outs.append(np.zeros(shape, dtype))
    n_params = len(in_names)
    n_outs = len(out_avals)
    all_names = in_names + out_names
    if partition_name is not None:
        all_names = all_names + [partition_name]
    donate = tuple(range(n_params, n_params + n_outs))

    def _body(*args):
        operands = list(args)
        if partition_name is not None:
            operands.append(bass2jax.partition_id_tensor())
        outs = bass2jax._bass_exec_p.bind(
            *operands,
            out_avals=tuple(out_avals),
            in_names=tuple(all_names),
            out_names=tuple(out_names),
            lowering_input_output_aliases=(),
            sim_require_finite=True,
            sim_require_nnan=True,
            nc=nc,
        )
        return tuple(outs)

    devices = jax.devices()[:NCORES]
    mesh = Mesh(np.asarray(devices), ("core",))
    sharded = jax.jit(
        shard_map(_body, mesh=mesh,
                  in_specs=(PartitionSpec("core"),) * (n_params + n_outs),
                  out_specs=(PartitionSpec("core"),) * n_outs,
                  check_rep=False),
        donate_argnums=donate, keep_unused=True)

    def execute(in_maps):
        concat_in = [
            np.concatenate([np.asarray(in_maps[c][nm]) for c in range(NCORES)], axis=0)
            for nm in in_names
        ]
        concat_zeros = [
            np.zeros((NCORES * z.shape[0], *z.shape[1:]), z.dtype) for z in zero_outs
        ]
        out_arrs = sharded(*concat_in, *concat_zeros)
        jax.block_until_ready(out_arrs)
        return [
            {nm: np.asarray(out_arrs[i]).reshape(NCORES, *out_avals[i].shape)[c]
             for i, nm in enumerate(out_names)}
            for c in range(NCORES)
        ]

    _cache[key] = execute
    return execute


class _Res:
    def __init__(self, results):
        self.results = results


def run(inputs, debug=False, trace=False):
    import ml_dtypes
    in_maps = host_inputs(inputs)
    for m in in_maps:
        for k in ("mask_sk", "tri", "ones128"):
            m[k] = m[k].astype(ml_dtypes.bfloat16)
    execute = _get_runner(debug=debug)
    return _Res(execute(in_maps)), in_maps


def kernel(**inputs) -> np.ndarray:
    res, _ = run(inputs)
    y = np.zeros((HID, S), np.float64)
    for c in range(NCORES):
        y += res.results[c]["ypart"].astype(np.float64)
    return y.T.reshape(B, S, HID).astype(np.float32)
